# revision 12
# baseline (speedup 1.0000x reference)
"""Trainium2 Bass kernel for nn_Convolution_24970939858998.

Conv2d: input [32, 8, 1024, 1024] f32, weight [8, 8, 3, 3], bias [8],
stride 1, pad 1 -> out [32, 8, 1024, 1024].

Strategy
--------
Data-parallel over batch: 4 images per core x 8 cores, no collectives.

Per core, the conv is computed as a *banded matmul*: for a block of 14
output rows, the 16 needed input rows (8 channels each -> K = 128 SBUF
partitions, p = r*8+ci) are multiplied by a host-prebuilt band weight
matrix lhsT[kw] of shape [128, 112] (m = dh*8+co, entry W[co,ci,r-dh,kw])
so a single PE pass produces all 8 output channels x 14 rows at once.
The kw=0..2 taps are 3 PSUM-accumulated matmuls whose rhs is the same
SBUF tile shifted by one column. PSUM -> SBUF copy fuses the per-channel
bias add (alternating ScalarE activation / VectorE tensor_scalar).

The kernel is HBM-bandwidth bound, so traffic is minimized two ways:
  * the input is laid out [h, c, b, w] host-side (w zero-padded) so each
    block's row load is one fully contiguous DMA;
  * both the input and the output cross HBM as bfloat16 (PSUM
    accumulation stays f32), halving traffic vs f32 in each direction.
    The f32 cast back happens host-side; rel-err stays ~1e-3, far under
    the 2e-2 gate.
"""

import os
import sys

import numpy as np
import ml_dtypes

for _p in ("/opt/trn_rl_repo", "/root/.axon_site/_ro/trn_rl_repo"):
    if os.path.isdir(_p) and _p not in sys.path:
        sys.path.insert(0, _p)
        break

import concourse.mybir as mybir
from concourse import bacc, bass_utils
from concourse.tile import TileContext

BF16 = ml_dtypes.bfloat16

B, C, CO, H, W = 32, 8, 8, 1024, 1024
KH = KW = 3
NCORES = 8
BPC = B // NCORES  # 4 images per core

RB = 14  # output rows per block
KR = 16  # input rows per block (RB + 2 halo)
M = CO * RB  # 112 output partitions (dh*8+co)
NW = 512  # w chunk (one PSUM bank of f32)

_PROG = None  # cached traced+compiled program
LAST_RESULTS = None  # bass_utils.BassKernelResults of the last run


def build_program(bpc=BPC, h=H, w=W):
    f32 = mybir.dt.float32
    bf16 = mybir.dt.bfloat16
    nblk = -(-h // RB)
    wp = w + 2
    nch = w // NW

    nc = bacc.Bacc("TRN2", debug=False)
    # input transposed on host: x[h, c, b, wp] (w zero-padded, h not)
    x = nc.dram_tensor("x", [h, C, bpc, wp], bf16, kind="ExternalInput").ap()
    wband = nc.dram_tensor("wband", [8 * KR, KW, M], bf16, kind="ExternalInput").ap()
    bias = nc.dram_tensor("bias", [M, 1], f32, kind="ExternalInput").ap()
    # output layout out[h, co, b, w], bf16 (upcast host-side)
    out = nc.dram_tensor("out", [h, CO, bpc, w], bf16, kind="ExternalOutput").ap()
    # one zero row-group for the top padding of block 0
    zrow = nc.dram_tensor("zrow", [8, bpc, wp], bf16, kind="ExternalInput").ap()

    with TileContext(nc) as tc:
        with (
            tc.tile_pool(name="const", bufs=1) as cpool,
            tc.tile_pool(name="xin", bufs=6) as xpool,
            tc.tile_pool(name="yout", bufs=6) as ypool,
            tc.tile_pool(name="acc", bufs=8, space="PSUM") as ppool,
        ):
            wt = cpool.tile([8 * KR, KW, M], bf16)
            bt = cpool.tile([M, 1], f32)

            xt_prev = None
            prev_h0 = None
            for j in range(nblk):
                h0 = j * RB
                nrows_out = min(RB, h - h0)
                if nrows_out < RB:
                    # partial last block: slide the window up so K stays full
                    # (small-K matmuls stream at half rate) and recompute rows
                    # already stored by the previous block; only the final
                    # nrows_out rows (partitions p0:M) are copied + stored
                    h0 = h - RB
                p0 = 8 * (RB - nrows_out)
                # tile partition p = r*8+ci holds padded row h0+r (= dram row
                # h0+r-1) of channel ci
                xt = xpool.tile([8 * KR, bpc, wp], bf16, tag="xt")
                if j == 0:
                    # the big row load goes out first on the sync ring; the
                    # small constants ride the scalar ring concurrently so the
                    # first matmul isn't serialized behind their descriptors
                    nc.sync.dma_start(
                        out=xt[8:128],
                        in_=x[0 : KR - 1].rearrange("r c b w -> (r c) b w"),
                    )
                    # padded row -1 = zeros from the zrow tensor (matmul base
                    # partitions must be 32-aligned, so K can't start at 8)
                    nc.scalar.dma_start(out=xt[0:8], in_=zrow)
                    nc.scalar.dma_start(out=wt, in_=wband)
                    nc.scalar.dma_start(out=bt, in_=bias)
                    k1 = 128
                else:
                    # 2-row halo rides on-chip from the previous block's tile
                    # (SBUF->SBUF, no HBM read); only fresh rows hit HBM
                    src = 8 * (h0 - 1 - (prev_h0 - 1))
                    nc.sync.dma_start(
                        out=xt[0:16], in_=xt_prev[src : src + 16]
                    )
                    hi = min(h0 + KR - 1, h)
                    nload = hi - (h0 + 1)
                    nc.sync.dma_start(
                        out=xt[16 : 16 + 8 * nload],
                        in_=x[h0 + 1 : hi].rearrange("r c b w -> (r c) b w"),
                    )
                    # rows past the image bottom stay unmaterialized: the
                    # contraction is truncated to the loaded partitions
                    k1 = 16 + 8 * nload
                xt_prev, prev_h0 = xt, h0

                yt = ypool.tile([M, bpc, w], bf16, tag="yt")
                for b in range(bpc):
                    for wc in range(nch):
                        w0 = wc * NW
                        ps = ppool.tile([M, NW], f32, tag="ps")
                        for i, kw in enumerate((1, 0, 2)):
                            nc.tensor.matmul(
                                ps,
                                wt[0:k1, kw, :],
                                xt[0:k1, b, w0 + kw : w0 + kw + NW],
                                start=(i == 0),
                                stop=(i == 2),
                            )
                        ysec = yt[:, b, w0 : w0 + NW]
                        # alternate the bank drain between the two copy
                        # engines; a single reader per bank — concurrent
                        # readers contend on the PSUM port and slow both
                        # engines and the accumulating matmuls
                        if (b + wc) % 2 == 0:
                            nc.scalar.add(ysec[p0:M], ps[p0:M], bt[p0:M])
                        else:
                            nc.vector.tensor_scalar_add(
                                ysec[p0:M], ps[p0:M], bt[p0:M]
                            )
                # stores on the second HWDGE ring (ACT) so their waits on the
                # bias copies never block load dispatch on the SP ring
                nc.scalar.dma_start(
                    out=out[h - nrows_out : h].rearrange("r c b w -> (r c) b w")
                    if p0
                    else out[h0 : h0 + RB].rearrange("r c b w -> (r c) b w"),
                    in_=yt[p0:M],
                )
    nc.compile()
    return nc


def pack_weights(weight: np.ndarray) -> np.ndarray:
    # lhsT[r*8+ci, kw, dh*8+co] = weight[co, ci, r-dh, kw] for 0 <= r-dh < 3
    wb = np.zeros((8 * KR, KW, M), np.float32)
    for dh in range(RB):
        for kh in range(KH):
            r = dh + kh
            wb[r * 8 : r * 8 + 8, :, dh * 8 : dh * 8 + 8] = weight[
                :, :, kh, :
            ].transpose(1, 2, 0)
    return wb.astype(BF16)


def pad_input(input, h, w):
    """input [n, C, h, w] f32 -> [h, C, n, w+2] bf16 (w zero-padded)."""
    n = input.shape[0]
    xpad = np.zeros((h, C, n, w + 2), BF16)
    xpad[:, :, :, 1 : 1 + w] = input.transpose(2, 1, 0, 3).astype(BF16)
    return xpad


def kernel(input, weight, bias):
    global _PROG, LAST_RESULTS
    input = np.asarray(input, dtype=np.float32)
    weight = np.asarray(weight, dtype=np.float32)
    bias = np.asarray(bias, dtype=np.float32)

    if _PROG is None:
        _PROG = build_program()
    nc = _PROG

    wb = pack_weights(weight)
    bias_m = np.tile(bias.astype(np.float32), RB).reshape(M, 1)

    zrow = np.zeros((8, BPC, W + 2), BF16)
    in_maps = [
        {
            "x": pad_input(input[c * BPC : (c + 1) * BPC], H, W),
            "wband": wb,
            "bias": bias_m,
            "zrow": zrow,
        }
        for c in range(NCORES)
    ]
    LAST_RESULTS = bass_utils.run_bass_kernel_spmd(
        nc, in_maps, core_ids=list(range(NCORES))
    )
    # out[h, co, b, w] bf16 -> [b, co, h, w] f32
    outs = [
        r["out"].astype(np.float32).transpose(2, 1, 0, 3) for r in LAST_RESULTS.results
    ]
    return np.concatenate(outs, axis=0)


# revision 14
# speedup vs baseline: 1.1928x; 1.1928x over previous
"""Trainium2 Bass kernel for nn_Convolution_24970939858998.

Conv2d: input [32, 8, 1024, 1024] f32, weight [8, 8, 3, 3], bias [8],
stride 1, pad 1 -> out [32, 8, 1024, 1024].

Strategy
--------
Data-parallel over batch: 4 images per core x 8 cores, no collectives.

Per core, the conv is computed as a *banded matmul*: for a block of 14
output rows, the 16 needed input rows (8 channels each -> K = 128 SBUF
partitions, p = r*8+ci) are multiplied by a host-prebuilt band weight
matrix lhsT[kw] of shape [128, 112] (m = dh*8+co, entry W[co,ci,r-dh,kw])
so a single PE pass produces all 8 output channels x 14 rows at once.
The kw=0..2 taps are 3 PSUM-accumulated matmuls whose rhs is the same
SBUF tile shifted by one column. PSUM -> SBUF copy fuses the per-channel
bias add (alternating ScalarE activation / VectorE tensor_scalar).

The kernel is HBM-bandwidth bound, so traffic is minimized two ways:
  * the input is laid out [h, c, b, w] host-side (w zero-padded) so each
    block's row load is one fully contiguous DMA;
  * both the input and the output cross HBM as bfloat16 (PSUM
    accumulation stays f32), halving traffic vs f32 in each direction.
    The f32 cast back happens host-side; rel-err stays ~1e-3, far under
    the 2e-2 gate.
"""

import os
import sys

import numpy as np
import ml_dtypes

for _p in ("/opt/trn_rl_repo", "/root/.axon_site/_ro/trn_rl_repo"):
    if os.path.isdir(_p) and _p not in sys.path:
        sys.path.insert(0, _p)
        break

import concourse.mybir as mybir
from concourse import bacc, bass_utils
from concourse.tile import TileContext

BF16 = ml_dtypes.bfloat16

B, C, CO, H, W = 32, 8, 8, 1024, 1024
KH = KW = 3
NCORES = 8
BPC = B // NCORES  # 4 images per core

RB = 14  # output rows per block
KR = 16  # input rows per block (RB + 2 halo)
M = CO * RB  # 112 output partitions (dh*8+co)
NW = 512  # w chunk (one PSUM bank of f32)

_PROG = None  # cached traced+compiled program
LAST_RESULTS = None  # bass_utils.BassKernelResults of the last run


def build_program(bpc=BPC, h=H, w=W):
    f32 = mybir.dt.float32
    bf16 = mybir.dt.bfloat16
    nblk = -(-h // RB)
    wp = w + 2
    nch = w // NW

    nc = bacc.Bacc("TRN2", debug=False)
    # input transposed on host: x[h, c, b, wp] (w zero-padded, h not)
    x = nc.dram_tensor("x", [h, C, bpc, wp], bf16, kind="ExternalInput").ap()
    wband = nc.dram_tensor("wband", [8 * KR, KW, M], bf16, kind="ExternalInput").ap()
    bias = nc.dram_tensor("bias", [M, 1], f32, kind="ExternalInput").ap()
    # output layout out[h, co, b, w], bf16 (upcast host-side)
    out = nc.dram_tensor("out", [h, CO, bpc, w], bf16, kind="ExternalOutput").ap()
    # one zero row-group for the top padding of block 0
    zrow = nc.dram_tensor("zrow", [8, bpc, wp], bf16, kind="ExternalInput").ap()

    with TileContext(nc) as tc:
        with (
            tc.tile_pool(name="const", bufs=1) as cpool,
            tc.tile_pool(name="xin", bufs=6) as xpool,
            tc.tile_pool(name="yout", bufs=6) as ypool,
            tc.tile_pool(name="acc", bufs=8, space="PSUM") as ppool,
        ):
            wt = cpool.tile([8 * KR, KW, M], bf16)
            bt = cpool.tile([M, 1], f32)

            for j in range(nblk):
                h0 = j * RB
                nrows_out = min(RB, h - h0)
                if nrows_out < RB:
                    # partial last block: slide the window up so K stays full
                    # (small-K matmuls stream at half rate) and recompute rows
                    # already stored by the previous block; only the final
                    # nrows_out rows (partitions p0:M) are copied + stored
                    h0 = h - RB
                p0 = 8 * (RB - nrows_out)
                # tile partition p = r*8+ci holds padded row h0+r (= dram row
                # h0+r-1) of channel ci
                xt = xpool.tile([8 * KR, bpc, wp], bf16, tag="xt")
                if j == 0:
                    # the big row load goes out first on the sync ring; the
                    # small constants ride the scalar ring concurrently so the
                    # first matmul isn't serialized behind their descriptors
                    nc.sync.dma_start(
                        out=xt[8:128],
                        in_=x[0 : KR - 1].rearrange("r c b w -> (r c) b w"),
                    )
                    # padded row -1 = zeros from the zrow tensor (matmul base
                    # partitions must be 32-aligned, so K can't start at 8)
                    nc.scalar.dma_start(out=xt[0:8], in_=zrow)
                    nc.scalar.dma_start(out=wt, in_=wband)
                    nc.scalar.dma_start(out=bt, in_=bias)
                    k1 = 128
                else:
                    # the 2 halo rows are re-read from HBM with the fresh rows
                    # (one fully contiguous load; an on-chip SBUF->SBUF halo
                    # copy serializes the load ring and loses badly)
                    lo = h0 - 1
                    hi = min(h0 + KR - 1, h)
                    nload = hi - lo
                    nc.sync.dma_start(
                        out=xt[0 : 8 * nload],
                        in_=x[lo:hi].rearrange("r c b w -> (r c) b w"),
                    )
                    # rows past the image bottom stay unmaterialized: the
                    # contraction is truncated to the loaded partitions
                    k1 = 8 * nload

                yt = ypool.tile([M, bpc, w], bf16, tag="yt")
                for b in range(bpc):
                    for wc in range(nch):
                        w0 = wc * NW
                        ps = ppool.tile([M, NW], f32, tag="ps")
                        for i, kw in enumerate((1, 0, 2)):
                            nc.tensor.matmul(
                                ps,
                                wt[0:k1, kw, :],
                                xt[0:k1, b, w0 + kw : w0 + kw + NW],
                                start=(i == 0),
                                stop=(i == 2),
                            )
                        ysec = yt[:, b, w0 : w0 + NW]
                        # alternate the bank drain between the two copy
                        # engines; a single reader per bank — concurrent
                        # readers contend on the PSUM port and slow both
                        # engines and the accumulating matmuls
                        if (b + wc) % 2 == 0:
                            nc.scalar.add(ysec[p0:M], ps[p0:M], bt[p0:M])
                        else:
                            nc.vector.tensor_scalar_add(
                                ysec[p0:M], ps[p0:M], bt[p0:M]
                            )
                # stores on the second HWDGE ring (ACT) so their waits on the
                # bias copies never block load dispatch on the SP ring
                nc.scalar.dma_start(
                    out=out[h - nrows_out : h].rearrange("r c b w -> (r c) b w")
                    if p0
                    else out[h0 : h0 + RB].rearrange("r c b w -> (r c) b w"),
                    in_=yt[p0:M],
                )
    nc.compile()
    return nc


def pack_weights(weight: np.ndarray) -> np.ndarray:
    # lhsT[r*8+ci, kw, dh*8+co] = weight[co, ci, r-dh, kw] for 0 <= r-dh < 3
    wb = np.zeros((8 * KR, KW, M), np.float32)
    for dh in range(RB):
        for kh in range(KH):
            r = dh + kh
            wb[r * 8 : r * 8 + 8, :, dh * 8 : dh * 8 + 8] = weight[
                :, :, kh, :
            ].transpose(1, 2, 0)
    return wb.astype(BF16)


def pad_input(input, h, w):
    """input [n, C, h, w] f32 -> [h, C, n, w+2] bf16 (w zero-padded)."""
    n = input.shape[0]
    xpad = np.zeros((h, C, n, w + 2), BF16)
    xpad[:, :, :, 1 : 1 + w] = input.transpose(2, 1, 0, 3).astype(BF16)
    return xpad


def kernel(input, weight, bias):
    global _PROG, LAST_RESULTS
    input = np.asarray(input, dtype=np.float32)
    weight = np.asarray(weight, dtype=np.float32)
    bias = np.asarray(bias, dtype=np.float32)

    if _PROG is None:
        _PROG = build_program()
    nc = _PROG

    wb = pack_weights(weight)
    bias_m = np.tile(bias.astype(np.float32), RB).reshape(M, 1)

    zrow = np.zeros((8, BPC, W + 2), BF16)
    in_maps = [
        {
            "x": pad_input(input[c * BPC : (c + 1) * BPC], H, W),
            "wband": wb,
            "bias": bias_m,
            "zrow": zrow,
        }
        for c in range(NCORES)
    ]
    LAST_RESULTS = bass_utils.run_bass_kernel_spmd(
        nc, in_maps, core_ids=list(range(NCORES))
    )
    # out[h, co, b, w] bf16 -> [b, co, h, w] f32
    outs = [
        r["out"].astype(np.float32).transpose(2, 1, 0, 3) for r in LAST_RESULTS.results
    ]
    return np.concatenate(outs, axis=0)


# revision 15
# speedup vs baseline: 1.2424x; 1.0416x over previous
"""Trainium2 Bass kernel for nn_Convolution_24970939858998.

Conv2d: input [32, 8, 1024, 1024] f32, weight [8, 8, 3, 3], bias [8],
stride 1, pad 1 -> out [32, 8, 1024, 1024].

Strategy
--------
Data-parallel over batch: 4 images per core x 8 cores, no collectives.

Per core, the conv is computed as a *banded matmul*: for a block of 14
output rows, the 16 needed input rows (8 channels each -> K = 128 SBUF
partitions, p = r*8+ci) are multiplied by a host-prebuilt band weight
matrix lhsT[kw] of shape [128, 112] (m = dh*8+co, entry W[co,ci,r-dh,kw])
so a single PE pass produces all 8 output channels x 14 rows at once.
The kw=0..2 taps are 3 PSUM-accumulated matmuls whose rhs is the same
SBUF tile shifted by one column. PSUM -> SBUF copy fuses the per-channel
bias add (alternating ScalarE activation / VectorE tensor_scalar).

The kernel is HBM-bandwidth bound, so traffic is minimized two ways:
  * the input is laid out [h, c, b, w] host-side (w zero-padded) so each
    block's row load is one fully contiguous DMA;
  * both the input and the output cross HBM as bfloat16 (PSUM
    accumulation stays f32), halving traffic vs f32 in each direction.
    The f32 cast back happens host-side; rel-err stays ~1e-3, far under
    the 2e-2 gate.
"""

import os
import sys

import numpy as np
import ml_dtypes

for _p in ("/opt/trn_rl_repo", "/root/.axon_site/_ro/trn_rl_repo"):
    if os.path.isdir(_p) and _p not in sys.path:
        sys.path.insert(0, _p)
        break

import concourse.mybir as mybir
from concourse import bacc, bass_utils
from concourse.tile import TileContext

BF16 = ml_dtypes.bfloat16

B, C, CO, H, W = 32, 8, 8, 1024, 1024
KH = KW = 3
NCORES = 8
BPC = B // NCORES  # 4 images per core

RB = 14  # output rows per block
KR = 16  # input rows per block (RB + 2 halo)
M = CO * RB  # 112 output partitions (dh*8+co)
NW = 512  # w chunk (one PSUM bank of f32)

_PROG = None  # cached traced+compiled program
LAST_RESULTS = None  # bass_utils.BassKernelResults of the last run


def build_program(bpc=BPC, h=H, w=W):
    f32 = mybir.dt.float32
    bf16 = mybir.dt.bfloat16
    nblk = -(-h // RB)
    wp = w + 2
    nch = w // NW

    nc = bacc.Bacc("TRN2", debug=False)
    # input transposed on host: x[h, c, b, wp] (w zero-padded, h not)
    x = nc.dram_tensor("x", [h, C, bpc, wp], bf16, kind="ExternalInput").ap()
    wband = nc.dram_tensor("wband", [8 * KR, KW, M], bf16, kind="ExternalInput").ap()
    bias = nc.dram_tensor("bias", [M, 1], f32, kind="ExternalInput").ap()
    # output layout out[h, co, b, w], bf16 (upcast host-side)
    out = nc.dram_tensor("out", [h, CO, bpc, w], bf16, kind="ExternalOutput").ap()
    # one zero row-group for the top padding of block 0
    zrow = nc.dram_tensor("zrow", [8, bpc, wp], bf16, kind="ExternalInput").ap()

    with TileContext(nc) as tc:
        with (
            tc.tile_pool(name="const", bufs=1) as cpool,
            tc.tile_pool(name="xin", bufs=6) as xpool,
            tc.tile_pool(name="yout", bufs=6) as ypool,
            tc.tile_pool(name="acc", bufs=8, space="PSUM") as ppool,
        ):
            wt = cpool.tile([8 * KR, KW, M], bf16)
            bt = cpool.tile([M, 1], f32)

            for j in range(nblk):
                h0 = j * RB
                nrows_out = min(RB, h - h0)
                if nrows_out < RB:
                    # partial last block: slide the window up so K stays full
                    # (small-K matmuls stream at half rate) and recompute rows
                    # already stored by the previous block; only the final
                    # nrows_out rows (partitions p0:M) are copied + stored
                    h0 = h - RB
                p0 = 8 * (RB - nrows_out)
                # tile partition p = r*8+ci holds padded row h0+r (= dram row
                # h0+r-1) of channel ci
                xt = xpool.tile([8 * KR, bpc, wp], bf16, tag="xt")
                if j == 0:
                    # the big row load goes out first on the sync ring; the
                    # small constants ride the scalar ring concurrently so the
                    # first matmul isn't serialized behind their descriptors
                    nc.sync.dma_start(
                        out=xt[8:128],
                        in_=x[0 : KR - 1].rearrange("r c b w -> (r c) b w"),
                    )
                    # padded row -1 = zeros from the zrow tensor (matmul base
                    # partitions must be 32-aligned, so K can't start at 8)
                    nc.scalar.dma_start(out=xt[0:8], in_=zrow)
                    nc.scalar.dma_start(out=wt, in_=wband)
                    nc.scalar.dma_start(out=bt, in_=bias)
                    k1 = 128
                else:
                    # the 2 halo rows are re-read from HBM with the fresh rows
                    # (one fully contiguous load; an on-chip SBUF->SBUF halo
                    # copy serializes the load ring and loses badly)
                    lo = h0 - 1
                    hi = min(h0 + KR - 1, h)
                    nload = hi - lo
                    nc.sync.dma_start(
                        out=xt[0 : 8 * nload],
                        in_=x[lo:hi].rearrange("r c b w -> (r c) b w"),
                    )
                    # rows past the image bottom stay unmaterialized: the
                    # contraction is truncated to the loaded partitions
                    k1 = 8 * nload

                yt = ypool.tile([M, bpc, w], bf16, tag="yt")
                for b in range(bpc):
                    for wc in range(nch):
                        w0 = wc * NW
                        ps = ppool.tile([M, NW], f32, tag="ps")
                        for i, kw in enumerate((1, 0, 2)):
                            nc.tensor.matmul(
                                ps,
                                wt[0:k1, kw, :],
                                xt[0:k1, b, w0 + kw : w0 + kw + NW],
                                start=(i == 0),
                                stop=(i == 2),
                            )
                        ysec = yt[:, b, w0 : w0 + NW]
                        # alternate the bank drain between the two copy
                        # engines; a single reader per bank — concurrent
                        # readers contend on the PSUM port and slow both
                        # engines and the accumulating matmuls
                        if (b + wc) % 2 == 0:
                            nc.scalar.add(ysec[p0:M], ps[p0:M], bt[p0:M])
                        else:
                            nc.vector.tensor_scalar_add(
                                ysec[p0:M], ps[p0:M], bt[p0:M]
                            )
                # stores ride the idle Pool sequencer's ring: their descriptor
                # generation then interleaves with nothing — the SP ring keeps
                # dispatching loads and the ACT/DVE sequencers only issue
                # PSUM-drain copies
                nc.gpsimd.dma_start(
                    out=out[h - nrows_out : h].rearrange("r c b w -> (r c) b w")
                    if p0
                    else out[h0 : h0 + RB].rearrange("r c b w -> (r c) b w"),
                    in_=yt[p0:M],
                )
    nc.compile()
    return nc


def pack_weights(weight: np.ndarray) -> np.ndarray:
    # lhsT[r*8+ci, kw, dh*8+co] = weight[co, ci, r-dh, kw] for 0 <= r-dh < 3
    wb = np.zeros((8 * KR, KW, M), np.float32)
    for dh in range(RB):
        for kh in range(KH):
            r = dh + kh
            wb[r * 8 : r * 8 + 8, :, dh * 8 : dh * 8 + 8] = weight[
                :, :, kh, :
            ].transpose(1, 2, 0)
    return wb.astype(BF16)


def pad_input(input, h, w):
    """input [n, C, h, w] f32 -> [h, C, n, w+2] bf16 (w zero-padded)."""
    n = input.shape[0]
    xpad = np.zeros((h, C, n, w + 2), BF16)
    xpad[:, :, :, 1 : 1 + w] = input.transpose(2, 1, 0, 3).astype(BF16)
    return xpad


def kernel(input, weight, bias):
    global _PROG, LAST_RESULTS
    input = np.asarray(input, dtype=np.float32)
    weight = np.asarray(weight, dtype=np.float32)
    bias = np.asarray(bias, dtype=np.float32)

    if _PROG is None:
        _PROG = build_program()
    nc = _PROG

    wb = pack_weights(weight)
    bias_m = np.tile(bias.astype(np.float32), RB).reshape(M, 1)

    zrow = np.zeros((8, BPC, W + 2), BF16)
    in_maps = [
        {
            "x": pad_input(input[c * BPC : (c + 1) * BPC], H, W),
            "wband": wb,
            "bias": bias_m,
            "zrow": zrow,
        }
        for c in range(NCORES)
    ]
    LAST_RESULTS = bass_utils.run_bass_kernel_spmd(
        nc, in_maps, core_ids=list(range(NCORES))
    )
    # out[h, co, b, w] bf16 -> [b, co, h, w] f32
    outs = [
        r["out"].astype(np.float32).transpose(2, 1, 0, 3) for r in LAST_RESULTS.results
    ]
    return np.concatenate(outs, axis=0)
